# revision 5
# baseline (speedup 1.0000x reference)
"""DGCNN (4x SAGEConv + SortPool + Conv1d + MLP) Trainium2 Bass kernel.

Sharding: data-parallel over the B=512 graphs -> 64 graphs per core on 8 cores.
Edges never cross graphs, so each core's message passing is local. The edge
list is converted on the host into a per-graph normalized adjacency
(AT[g][s,d] = multiplicity(s->d) / max(deg(d),1)); aggregation then becomes a
block-diagonal dense matmul on the PE array (2 graphs of 64 nodes per
128-partition tile).

The SAGE stack runs in strict fp32: the SortPool keys (feature 255 of layer 3)
have top-30 gaps as small as 5.7e-7 on this data, so any matmul rounding on
that path (f32r keeps only ~11 mantissa bits) would reorder the selection and
blow the error gate. Everything downstream of the selection (one-hot gather,
conv1d, lin1) tolerates bf16, which runs the PE at 4x the fp32 rate.

SortPool is computed exactly (stable argsort semantics incl. ties) via a rank
computation: rank(i) = #{j : k_j > k_i} on keys perturbed by k_i -= i*1e-11.
Selection of the top-30 rows per graph is a one-hot bf16 matmul.

Conv1d is 8 accumulated bf16 matmuls per output tile (im2col via strided
access patterns, never materialized). lin1 weights are preloaded bf16.
"""

import numpy as np
import ml_dtypes

import concourse.bass as bass
import concourse.bacc as bacc
import concourse.mybir as mybir
import concourse.tile as tile
from concourse.bass_utils import run_bass_kernel_spmd

B, P, K, KS = 512, 64, 30, 4
N, E, F, H = B * P, 524288, 128, 256
L_OUT = K - KS + 1          # 27
N_CLASSES = 10
N_CORES = 8
GPC = B // N_CORES          # 64 graphs / core
NPC = GPC * P               # 4096 nodes / core
PAIRS = GPC // 2            # 32 pair-tiles (2 graphs of 64 nodes = 128 partitions)
NCHUNK = 512                # free-dim chunk for weight matmuls (4 pairs)
CPAIRS = NCHUNK // 128      # pairs per chunk
NCH = NPC // NCHUNK         # 8 chunks
F32 = mybir.dt.float32
BF16 = mybir.dt.bfloat16
EPS_TIE = 1e-11

NLAYERS = 4
GCHUNK = 16                 # graphs per conv psum tile (16*28 = 448 <= 512)
L28 = L_OUT + 1             # conv free dim padded even
TKPAD = GPC * K + 8         # topkT free size incl. zeroed overrun pad
S1 = 2 * L_OUT              # 54 lin1 contraction steps of 128
BF = np.dtype(ml_dtypes.bfloat16)


# ---------------------------------------------------------------- host prep

def _prep_shared(inp):
    """Host-side weight/constant reshaping (identical for every core)."""
    sh = {}
    for li in range(4):
        sh[f"wl{li}"] = np.ascontiguousarray(inp[f"sage{li}_wl"], np.float32)
        sh[f"wr{li}"] = np.ascontiguousarray(inp[f"sage{li}_wr"], np.float32)
        sh[f"b{li}"] = np.ascontiguousarray(inp[f"sage{li}_b"], np.float32)
    w = np.asarray(inp["conv1d_w"], np.float32)            # [O=256, I=256, KS]
    w2 = np.empty((2 * KS, 128, H), np.float32)
    for k in range(KS):
        wt = w[:, :, k].T                                  # [I, O]
        for ih in range(2):
            w2[k * 2 + ih] = wt[ih * 128:(ih + 1) * 128]
    sh["w2"] = w2.astype(BF)
    sh["cb"] = np.ascontiguousarray(inp["conv1d_b"], np.float32)
    w1 = np.asarray(inp["lin1_w"], np.float32)             # [6912, 256]
    sh["w1"] = np.ascontiguousarray(
        w1.reshape(2, 128, L_OUT, H).transpose(0, 2, 1, 3).reshape(S1, 128, H)
    ).astype(BF)
    sh["lb1"] = np.ascontiguousarray(
        np.broadcast_to(np.asarray(inp["lin1_b"], np.float32), (GPC, H)))
    sh["w4"] = np.ascontiguousarray(inp["lin2_w"], np.float32)   # [256, 128]
    sh["b2q"] = np.ascontiguousarray(inp["lin2_b"], np.float32)  # [128]
    sh["w5"] = np.ascontiguousarray(inp["out_w"], np.float32)    # [128, 10]
    sh["b3q"] = np.asarray(inp["out_b"], np.float32).reshape(N_CLASSES, 1).copy()
    sh["iota60"] = np.ascontiguousarray(
        np.broadcast_to(np.arange(2 * K, dtype=np.float32), (128, 2 * K)))
    off30 = np.zeros((128, 1), np.float32)
    off30[64:] = float(K)
    sh["off30"] = off30
    sh["epsrow"] = np.ascontiguousarray(
        np.broadcast_to(np.arange(P, dtype=np.float32) * np.float32(EPS_TIE), (P, P))).astype(np.float32)
    sh["id128"] = np.eye(128, dtype=np.float32)
    return sh


def _prep_cores(inp):
    """Per-core shards: node features (plain + transposed) and blockdiag adjacency."""
    x = np.nan_to_num(np.asarray(inp["x"], np.float32))
    ei = np.asarray(inp["edge_index"])
    src = ei[0].astype(np.int64)
    dst = ei[1].astype(np.int64)
    deg = np.bincount(dst, minlength=N).astype(np.float32)
    inv_deg = (1.0 / np.maximum(deg, 1.0)).astype(np.float32)
    g = src // P
    flat = g * (P * P) + (src % P) * P + (dst % P)
    AT = np.bincount(flat, minlength=B * P * P).astype(np.float32).reshape(B, P, P)
    AT *= inv_deg.reshape(B, P)[:, None, :]

    cores = []
    for c in range(N_CORES):
        xc = np.ascontiguousarray(x[c * NPC:(c + 1) * NPC])          # [4096, 128]
        atbd = np.zeros((PAIRS, 128, 128), np.float32)
        for t in range(PAIRS):
            atbd[t, :P, :P] = AT[c * GPC + 2 * t]
            atbd[t, P:, P:] = AT[c * GPC + 2 * t + 1]
        cores.append({
            "x": xc,
            "xt": np.ascontiguousarray(xc.T),                        # [128, 4096]
            "atbd": atbd,
        })
    return cores


# ---------------------------------------------------------------- device kernel

def _build(nc):
    """Emit the whole per-core kernel under a TileContext."""
    dt = nc.dram_tensor
    d_x = dt("x", [NPC, F], F32, kind="ExternalInput")
    d_xt = dt("xt", [F, NPC], F32, kind="ExternalInput")
    d_atbd = dt("atbd", [PAIRS, 128, 128], F32, kind="ExternalInput")
    d_wl, d_wr, d_b = [], [], []
    for li in range(4):
        fin = F if li == 0 else H
        d_wl.append(dt(f"wl{li}", [fin, H], F32, kind="ExternalInput"))
        d_wr.append(dt(f"wr{li}", [fin, H], F32, kind="ExternalInput"))
        d_b.append(dt(f"b{li}", [H], F32, kind="ExternalInput"))
    d_w2 = dt("w2", [2 * KS, 128, H], BF16, kind="ExternalInput")
    d_cb = dt("cb", [H], F32, kind="ExternalInput")
    d_w1 = dt("w1", [S1, 128, H], BF16, kind="ExternalInput")
    d_lb1 = dt("lb1", [GPC, H], F32, kind="ExternalInput")
    d_w4 = dt("w4", [H, 128], F32, kind="ExternalInput")
    d_b2q = dt("b2q", [128], F32, kind="ExternalInput")
    d_w5 = dt("w5", [128, N_CLASSES], F32, kind="ExternalInput")
    d_b3q = dt("b3q", [N_CLASSES, 1], F32, kind="ExternalInput")
    d_iota60 = dt("iota60", [128, 2 * K], F32, kind="ExternalInput")
    d_off30 = dt("off30", [128, 1], F32, kind="ExternalInput")
    d_epsrow = dt("epsrow", [P, P], F32, kind="ExternalInput")
    d_id128 = dt("id128", [128, 128], F32, kind="ExternalInput")
    d_out = dt("out", [N_CLASSES, GPC], F32, kind="ExternalOutput")

    with tile.TileContext(nc) as tc:
        _emit(tc, nc, locals())
    nc.compile()
    return nc


def _ap(base, extra_offset, free_dims):
    """Build a custom AP view: keep base's partition dim, replace free dims."""
    return bass.AP(base.tensor, base.offset + extra_offset,
                   [base.ap[0]] + list(free_dims))


def _emit(tc, nc, d):
    from contextlib import ExitStack
    ctx = ExitStack()
    with ctx:
        persist = ctx.enter_context(tc.tile_pool(name="persist", bufs=1))
        act_pool = ctx.enter_context(tc.tile_pool(name="acts", bufs=1))
        w1_pool = ctx.enter_context(tc.tile_pool(name="w1p", bufs=1))

        # ---- persistent loads (weight DMAs deferred until after input DMAs)
        _deferred = []

        def load(name, shape, view=None, dram=None, dtype=F32):
            t = persist.tile(shape, dtype, tag=name)
            src = (dram if dram is not None else d[f"d_{name}"]).ap()
            if view is not None:
                src = src.rearrange(*view[0], **view[1])
            _deferred.append((t, src))
            return t

        wl, wr, bias = [], [], []
        for li in range(4):
            ki = 1 if li == 0 else 2
            wl.append(load(f"wl{li}", [128, ki, H], (["(k p) o -> p k o"], {"p": 128}),
                           dram=d["d_wl"][li]))
            wr.append(load(f"wr{li}", [128, ki, H], (["(k p) o -> p k o"], {"p": 128}),
                           dram=d["d_wr"][li]))
            bias.append(load(f"b{li}", [128, 2], (["(h p) -> p h"], {"p": 128}),
                             dram=d["d_b"][li]))
        w2 = load("w2", [128, 2 * KS, H], (["k p o -> p k o"], {}), dtype=BF16)
        cb = load("cb", [128, 2], (["(h p) -> p h"], {"p": 128}))
        b1 = load("lb1", [GPC, H])
        w4 = load("w4", [128, 2, 128], (["(k p) o -> p k o"], {"p": 128}))
        b2q = load("b2q", [128, 1])
        w5 = load("w5", [128, N_CLASSES])
        b3q = load("b3q", [N_CLASSES, 1])
        iota60 = load("iota60", [128, 2 * K])
        off30 = load("off30", [128, 1])
        epsrow = load("epsrow", [P, P])
        id128 = load("id128", [128, 128])

        # lin1 weights, preloaded bf16 (DMA issued on the gpsimd queue after
        # the input slices so it doesn't delay the first matmuls)
        w1blk = w1_pool.tile([128, S1, H], BF16, tag="w1blk")

        # ---- activations
        h_sb = act_pool.tile([128, PAIRS, H], F32, tag="h")       # nodes on partitions
        hT_a = act_pool.tile([128, 2, NPC], F32, tag="hTa")
        hT_b = act_pool.tile([128, 2, NPC], F32, tag="hTb")
        hts = [hT_a, hT_b]

        # ---- input DMAs spread over the 3 DMA-capable queues (sync, scalar,
        # gpsimd); x lands in h_sb[:, :, 0:128]
        qs = [nc.sync, nc.scalar, nc.gpsimd]
        with tc.tile_pool(name="sage", bufs=1) as sg:
            atbd_parts = []
            for g in range(4):
                src = d["d_x"].ap().rearrange("(t p) f -> p t f", p=128)[:, g * 8:(g + 1) * 8, :]
                qs[g % 3].dma_start(h_sb[:, g * 8:(g + 1) * 8, 0:F], src)
                t_at = sg.tile([128, PAIRS // 4, 128], F32, tag=f"atbd{g}",
                               name=f"atbd{g}")
                srca = d["d_atbd"].ap().rearrange("t p n -> p t n")[:, g * 8:(g + 1) * 8, :]
                qs[(g + 1) % 3].dma_start(t_at[...], srca)
                atbd_parts.append(t_at)
                qs[(g + 2) % 3].dma_start(
                    hT_a[:, 0, g * 1024:(g + 1) * 1024],
                    d["d_xt"].ap()[:, g * 1024:(g + 1) * 1024])

            for _i, (_t, _src) in enumerate(_deferred):
                (nc.sync if _i % 2 == 0 else nc.scalar).dma_start(_t[...], _src)
            _deferred.clear()
            nc.gpsimd.dma_start(
                w1blk[...], d["d_w1"].ap().rearrange("s p o -> p s o"))

            # ---- SAGE layers, chunk-interleaved (agg -> weights -> transpose)
            with tc.tile_pool(name="aggdb", bufs=2) as adb, \
                 tc.tile_pool(name="ps_sage", bufs=2, space="PSUM") as psa, \
                 tc.tile_pool(name="ps_w", bufs=2, space="PSUM") as psw, \
                 tc.tile_pool(name="ps_tr", bufs=2, space="PSUM") as pst:
                for li in range(NLAYERS):
                    ki = 1 if li == 0 else 2
                    hTv = hts[li % 2]                    # prev layer's hT (L0: xT half)
                    hTo = hts[(li + 1) % 2]              # this layer's output
                    for c in range(NCH):
                        at = adb.tile([128, ki, NCHUNK], F32, tag="aggT")
                        for tp in range(CPAIRS):
                            t = CPAIRS * c + tp
                            for mh in range(ki):
                                ps = psa.tile([128, 128], F32, tag="psa")
                                lhsT = (h_sb[:, t, 0:F] if li == 0
                                        else h_sb[:, t, mh * 128:(mh + 1) * 128])
                                nc.tensor.matmul(
                                    ps[...],
                                    lhsT=lhsT,
                                    rhs=atbd_parts[t // 8][:, t % 8, :],
                                    start=True, stop=True)
                                nc.any.tensor_copy(
                                    at[:, mh, tp * 128:(tp + 1) * 128], ps[...])
                        sl = slice(c * NCHUNK, (c + 1) * NCHUNK)
                        for oh in range(2):
                            ps = psw.tile([128, NCHUNK], F32, tag="psw")
                            step, nsteps = 0, 2 * ki
                            for wmat, rt in ((wl[li], at), (wr[li], hTv)):
                                for kh in range(ki):
                                    rhs = (at[:, kh, :] if rt is at
                                           else hTv[:, kh, sl])
                                    nc.tensor.matmul(
                                        ps[...],
                                        lhsT=wmat[:, kh, oh * 128:(oh + 1) * 128],
                                        rhs=rhs,
                                        start=(step == 0), stop=(step == nsteps - 1))
                                    step += 1
                            nc.scalar.activation(
                                hTo[:, oh, sl], ps[...],
                                mybir.ActivationFunctionType.Relu,
                                bias=bias[li][:, oh:oh + 1])
                        if li < NLAYERS - 1:
                            # h_next = transpose(hT_next) per pair (PE transpose)
                            for tp in range(CPAIRS):
                                t = CPAIRS * c + tp
                                for oh in range(2):
                                    ps = pst.tile([128, 128], F32, tag="pst")
                                    nc.tensor.transpose(
                                        ps[...], hTo[:, oh, t * 128:(t + 1) * 128],
                                        id128[...])
                                    nc.any.tensor_copy(
                                        h_sb[:, t, oh * 128:(oh + 1) * 128], ps[...])

        # ---------------- tail: sort, selection, conv, mlp
        hT4 = hts[NLAYERS % 2]
        with tc.tile_pool(name="tail", bufs=1) as tp_:
            h3bf = tp_.tile([128, PAIRS, H], BF16, tag="h3bf")
            pt_all = tp_.tile([128, PAIRS, 2 * K], BF16, tag="pt")
            topkT = tp_.tile([128, 2, TKPAD], BF16, tag="topkT")

            with tc.tile_pool(name="ps_tr2", bufs=2, space="PSUM") as pst2:
                # ---- sort: ranks of the last feature channel per graph
                with tc.tile_pool(name="sort_scratch", bufs=1) as ss:
                    km = ss.tile([P, P], F32, tag="km")
                    # keys: feature 255 = (hi=1, p=127); node n = g*64+i
                    nc.sync.dma_start(km[...], hT4[127:128, 1, :])
                    kmp = ss.tile([P, P], F32, tag="kmp")
                    nc.vector.tensor_sub(kmp[...], km[...], epsrow[...])
                    cbt = ss.tile([P, P * P], BF16, tag="cbt")
                    kb = kmp[:, :]
                    in0 = _ap(kb, 0, [[0, P], kb.ap[1]])   # [g, i(bc), j]  k(g, j)
                    in1 = _ap(kb, 0, [kb.ap[1], [0, P]])   # [g, i, j(bc)]  k(g, i)
                    nc.vector.tensor_tensor(
                        _ap(cbt[:, :], 0, [[P, P], [1, P]]), in0, in1,
                        op=mybir.AluOpType.is_gt)
                    rk = ss.tile([P, P], F32, tag="rk")
                    nc.vector.tensor_reduce(
                        rk[...], _ap(cbt[:, :], 0, [[P, P], [1, P]]),
                        axis=mybir.AxisListType.X, op=mybir.AluOpType.add)
                    # transpose ranks -> [node i, graph g]
                    rt = ss.tile([P, P], F32, tag="rt")
                    pr = pst2.tile([P, P], F32, tag="pr")
                    nc.tensor.transpose(pr[...], rk[...], id128[0:P, 0:P])
                    nc.any.tensor_copy(rt[...], pr[...])
                    # rankP[p, t] = rank(node p%64 of graph 2t + p//64)
                    rankp = tp_.tile([128, PAIRS], F32, tag="rankp")
                    rb = rt[:, :]
                    nc.vector.tensor_copy(rankp[0:P, :], _ap(rb, 0, [[2, PAIRS]]))
                    nc.sync.dma_start(rankp[P:128, :], _ap(rb, 1, [[2, PAIRS]]))
                    # rank2 = rankp + 30*(p>=64) + 1000*(rankp>=30)
                    ge30 = tp_.tile([128, PAIRS], F32, tag="ge30")
                    nc.vector.tensor_scalar(ge30[...], rankp[...], float(K), None,
                                            op0=mybir.AluOpType.is_ge)
                    rank2 = tp_.tile([128, PAIRS], F32, tag="rank2")
                    nc.vector.scalar_tensor_tensor(
                        rank2[...], ge30[...], 1000.0,
                        rankp[...], op0=mybir.AluOpType.mult,
                        op1=mybir.AluOpType.add)
                    nc.vector.tensor_scalar(rank2[...], rank2[...],
                                            off30[:, 0:1], None,
                                            op0=mybir.AluOpType.add)
                    # one-hot selection matrices  PT[p, t, c] = (c == rank2[p, t])
                    io = iota60[:, :]
                    r2 = rank2[:, :]
                    nc.vector.tensor_tensor(
                        pt_all[...],
                        _ap(io, 0, [[0, PAIRS], [1, 2 * K]]),
                        _ap(r2, 0, [[1, PAIRS], [0, 2 * K]]),
                        op=mybir.AluOpType.is_equal)

                # ---- L3 transposes -> bf16 node-major copy for selection
                for t in range(PAIRS):
                    for oh in range(2):
                        ps = pst2.tile([128, 128], F32, tag="pst2")
                        nc.tensor.transpose(
                            ps[...], hT4[:, oh, t * 128:(t + 1) * 128], id128[...])
                        nc.any.tensor_copy(
                            h3bf[:, t, oh * 128:(oh + 1) * 128], ps[...])

            with tc.tile_pool(name="convt", bufs=1) as cv:
                y_sb = cv.tile([128, 2, GPC, L28], BF16, tag="y")
                with tc.tile_pool(name="ps_tail", bufs=2, space="PSUM") as ptl:
                    # ---- selection: topkT[f, b*30+r] = sum_n h3[n,f] PT[n,b,r]
                    nc.vector.memset(topkT[:, :, GPC * K:], 0.0)
                    for t in range(PAIRS):
                        for mh in range(2):
                            ps = ptl.tile([128, 2 * K], F32, tag="pssel")
                            nc.tensor.matmul(
                                ps[...],
                                lhsT=h3bf[:, t, mh * 128:(mh + 1) * 128],
                                rhs=pt_all[:, t, :],
                                start=True, stop=True)
                            nc.any.tensor_copy(
                                topkT[:, mh, t * 2 * K:(t + 1) * 2 * K], ps[...])

                    # conv1d: y[p,oh,b,l] = relu(sum_{k,ih} w2^T topkT[...] + cb)
                    for oh in range(2):
                        for bc in range(GPC // GCHUNK):
                            ps = ptl.tile([128, GCHUNK, L28], F32, tag="psconv")
                            step = 0
                            for k in range(KS):
                                for ih in range(2):
                                    base = topkT[:, ih, :]
                                    rhs = _ap(base, bc * GCHUNK * K + k,
                                              [[K, GCHUNK], [1, L28]])
                                    nc.tensor.matmul(
                                        ps[...],
                                        lhsT=w2[:, k * 2 + ih,
                                                oh * 128:(oh + 1) * 128],
                                        rhs=rhs,
                                        start=(step == 0),
                                        stop=(step == 2 * KS - 1))
                                    step += 1
                            nc.scalar.activation(
                                y_sb[:, oh, bc * GCHUNK:(bc + 1) * GCHUNK, :],
                                ps[...],
                                mybir.ActivationFunctionType.Relu,
                                bias=cb[:, oh:oh + 1])

                with tc.tile_pool(name="ps_fin", bufs=1, space="PSUM") as pfin:
                    # lin1 (b-major): z1T[b, o] = relu(sum_s y_s^T @ w1_s + b1)
                    ps1 = pfin.tile([GPC, H], F32, tag="ps1")
                    for s in range(S1):
                        ot, l = divmod(s, L_OUT)
                        nc.tensor.matmul(
                            ps1[...],
                            lhsT=y_sb[:, ot, :, l],
                            rhs=w1blk[:, s, :],
                            start=(s == 0), stop=(s == S1 - 1))
                    z1t = cv.tile([GPC, H], F32, tag="z1t")
                    nc.vector.tensor_add(z1t[...], ps1[...], b1[...])
                    nc.scalar.activation(z1t[...], z1t[...],
                                         mybir.ActivationFunctionType.Relu,
                                         bias=0.0)
                    # transpose z1T -> z1 [o on partitions]
                    z1 = cv.tile([128, 2, GPC], F32, tag="z1")
                    for mh in range(2):
                        psz = pfin.tile([128, GPC], F32, tag="psz")
                        nc.tensor.transpose(psz[...],
                                            z1t[:, mh * 128:(mh + 1) * 128],
                                            id128[0:GPC, 0:GPC])
                        nc.any.tensor_copy(z1[:, mh, :], psz[...])

                    # lin2 + out
                    ps2 = pfin.tile([128, GPC], F32, tag="ps2")
                    for kh in range(2):
                        nc.tensor.matmul(ps2[...], lhsT=w4[:, kh, :],
                                         rhs=z1[:, kh, :],
                                         start=(kh == 0), stop=(kh == 1))
                    z2 = cv.tile([128, GPC], F32, tag="z2")
                    nc.scalar.activation(z2[...], ps2[...],
                                         mybir.ActivationFunctionType.Relu,
                                         bias=b2q[:, 0:1])
                    ps3 = pfin.tile([N_CLASSES, GPC], F32, tag="ps3")
                    nc.tensor.matmul(ps3[...], lhsT=w5[...], rhs=z2[...],
                                     start=True, stop=True)
                    o_sb = cv.tile([N_CLASSES, GPC], F32, tag="osb")
                    nc.scalar.activation(o_sb[...], ps3[...],
                                         mybir.ActivationFunctionType.Relu,
                                         bias=b3q[:, 0:1])
                    nc.sync.dma_start(d["d_out"].ap(), o_sb[...])


# ---------------------------------------------------------------- entry point

_CACHED = {}


def _get_nc():
    if "nc" not in _CACHED:
        nc = bacc.Bacc("TRN2", target_bir_lowering=False, debug=False,
                       enable_asserts=True)
        _CACHED["nc"] = _build(nc)
    return _CACHED["nc"]


def make_in_maps(inputs):
    sh = _prep_shared(inputs)
    cores = _prep_cores(inputs)
    return [{**sh, **c} for c in cores]


TRACE = False


def kernel(**inputs):
    in_maps = make_in_maps(inputs)
    nc = _get_nc()
    res = run_bass_kernel_spmd(nc, in_maps, core_ids=list(range(N_CORES)),
                               trace=TRACE)
    _CACHED["last_res"] = res
    return np.concatenate([np.asarray(r["out"], np.float32).T
                           for r in res.results], axis=0)


if __name__ == "__main__":
    import reference
    inputs = {k: np.asarray(v) for k, v in reference.setup_inputs().items()}
    out = kernel(**inputs)
    print("out", out.shape, out.dtype)


# revision 11
# speedup vs baseline: 1.0755x; 1.0755x over previous
"""DGCNN (4x SAGEConv + SortPool + Conv1d + MLP) Trainium2 Bass kernel.

Sharding: data-parallel over the B=512 graphs -> 64 graphs per core on 8 cores.
Edges never cross graphs, so each core's message passing is local. The edge
list is converted on the host into a per-graph normalized adjacency
(AT[g][s,d] = multiplicity(s->d) / max(deg(d),1)); aggregation then becomes a
block-diagonal dense matmul on the PE array (2 graphs of 64 nodes per
128-partition tile).

The SAGE stack runs in strict fp32: the SortPool keys (feature 255 of layer 3)
have top-30 gaps as small as 5.7e-7 on this data, so any matmul rounding on
that path (f32r keeps only ~11 mantissa bits) would reorder the selection and
blow the error gate. Everything downstream of the selection (one-hot gather,
conv1d, lin1) tolerates bf16, which runs the PE at 4x the fp32 rate.

SortPool is computed exactly (stable argsort semantics incl. ties) via a rank
computation: rank(i) = #{j : k_j > k_i} on keys perturbed by k_i -= i*1e-11.
Selection of the top-30 rows per graph is a one-hot bf16 matmul.

Conv1d is 8 accumulated bf16 matmuls per output tile (im2col via strided
access patterns, never materialized). lin1 weights are preloaded bf16.
"""

import numpy as np
import ml_dtypes

import concourse.bass as bass
import concourse.bacc as bacc
import concourse.mybir as mybir
import concourse.tile as tile
from concourse.bass_utils import run_bass_kernel_spmd

B, P, K, KS = 512, 64, 30, 4
N, E, F, H = B * P, 524288, 128, 256
L_OUT = K - KS + 1          # 27
N_CLASSES = 10
N_CORES = 8
GPC = B // N_CORES          # 64 graphs / core
NPC = GPC * P               # 4096 nodes / core
PAIRS = GPC // 2            # 32 pair-tiles (2 graphs of 64 nodes = 128 partitions)
NCHUNK = 512                # free-dim chunk for weight matmuls (4 pairs)
CPAIRS = NCHUNK // 128      # pairs per chunk
NCH = NPC // NCHUNK         # 8 chunks
W1PRE = 32                  # lin1 weight tiles preloaded during SAGE (rest streamed)
F32 = mybir.dt.float32
BF16 = mybir.dt.bfloat16
EPS_TIE = 1e-11

NLAYERS = 4
GCHUNK = 16                 # graphs per conv psum tile (16*28 = 448 <= 512)
L28 = L_OUT + 1             # conv free dim padded even
TKPAD = GPC * K + 8         # topkT free size incl. zeroed overrun pad
S1 = 2 * L_OUT              # 54 lin1 contraction steps of 128
BF = np.dtype(ml_dtypes.bfloat16)


# ---------------------------------------------------------------- host prep

def _prep_shared(inp):
    """Host-side weight/constant reshaping (identical for every core)."""
    sh = {}
    for li in range(4):
        sh[f"wl{li}"] = np.ascontiguousarray(inp[f"sage{li}_wl"], np.float32)
        sh[f"wr{li}"] = np.ascontiguousarray(inp[f"sage{li}_wr"], np.float32)
        sh[f"b{li}"] = np.ascontiguousarray(inp[f"sage{li}_b"], np.float32)
    w = np.asarray(inp["conv1d_w"], np.float32)            # [O=256, I=256, KS]
    w2 = np.empty((2 * KS, 128, H), np.float32)
    for k in range(KS):
        wt = w[:, :, k].T                                  # [I, O]
        for ih in range(2):
            w2[k * 2 + ih] = wt[ih * 128:(ih + 1) * 128]
    sh["w2"] = w2.astype(BF)
    sh["cb"] = np.ascontiguousarray(inp["conv1d_b"], np.float32)
    w1 = np.asarray(inp["lin1_w"], np.float32)             # [6912, 256]
    sh["w1"] = np.ascontiguousarray(
        w1.reshape(2, 128, L_OUT, H).transpose(0, 2, 1, 3).reshape(S1, 128, H)
    ).astype(BF)
    sh["lb1"] = np.ascontiguousarray(
        np.broadcast_to(np.asarray(inp["lin1_b"], np.float32), (GPC, H)))
    sh["w4"] = np.ascontiguousarray(inp["lin2_w"], np.float32)   # [256, 128]
    sh["b2q"] = np.ascontiguousarray(inp["lin2_b"], np.float32)  # [128]
    sh["w5"] = np.ascontiguousarray(inp["out_w"], np.float32)    # [128, 10]
    sh["b3q"] = np.asarray(inp["out_b"], np.float32).reshape(N_CLASSES, 1).copy()
    sh["iota60"] = np.ascontiguousarray(
        np.broadcast_to(np.arange(2 * K, dtype=np.float32), (128, 2 * K)))
    off30 = np.zeros((128, 1), np.float32)
    off30[64:] = float(K)
    sh["off30"] = off30
    sh["epsrow"] = np.ascontiguousarray(
        np.broadcast_to(np.arange(P, dtype=np.float32) * np.float32(EPS_TIE), (P, P))).astype(np.float32)
    sh["id128"] = np.eye(128, dtype=np.float32)
    return sh


def _prep_cores(inp):
    """Per-core shards: node features (plain + transposed) and blockdiag adjacency."""
    x = np.nan_to_num(np.asarray(inp["x"], np.float32))
    ei = np.asarray(inp["edge_index"])
    src = ei[0].astype(np.int64)
    dst = ei[1].astype(np.int64)
    deg = np.bincount(dst, minlength=N).astype(np.float32)
    inv_deg = (1.0 / np.maximum(deg, 1.0)).astype(np.float32)
    g = src // P
    flat = g * (P * P) + (src % P) * P + (dst % P)
    AT = np.bincount(flat, minlength=B * P * P).astype(np.float32).reshape(B, P, P)
    AT *= inv_deg.reshape(B, P)[:, None, :]

    cores = []
    for c in range(N_CORES):
        xc = np.ascontiguousarray(x[c * NPC:(c + 1) * NPC])          # [4096, 128]
        atbd = np.zeros((PAIRS, 128, 128), np.float32)
        for t in range(PAIRS):
            atbd[t, :P, :P] = AT[c * GPC + 2 * t]
            atbd[t, P:, P:] = AT[c * GPC + 2 * t + 1]
        cores.append({
            "x": xc,
            "xt": np.ascontiguousarray(xc.T),                        # [128, 4096]
            "atbd": atbd,
        })
    return cores


# ---------------------------------------------------------------- device kernel

def _build(nc):
    """Emit the whole per-core kernel under a TileContext."""
    dt = nc.dram_tensor
    d_x = dt("x", [NPC, F], F32, kind="ExternalInput")
    d_xt = dt("xt", [F, NPC], F32, kind="ExternalInput")
    d_atbd = dt("atbd", [PAIRS, 128, 128], F32, kind="ExternalInput")
    d_wl, d_wr, d_b = [], [], []
    for li in range(4):
        fin = F if li == 0 else H
        d_wl.append(dt(f"wl{li}", [fin, H], F32, kind="ExternalInput"))
        d_wr.append(dt(f"wr{li}", [fin, H], F32, kind="ExternalInput"))
        d_b.append(dt(f"b{li}", [H], F32, kind="ExternalInput"))
    d_w2 = dt("w2", [2 * KS, 128, H], BF16, kind="ExternalInput")
    d_cb = dt("cb", [H], F32, kind="ExternalInput")
    d_w1 = dt("w1", [S1, 128, H], BF16, kind="ExternalInput")
    d_lb1 = dt("lb1", [GPC, H], F32, kind="ExternalInput")
    d_w4 = dt("w4", [H, 128], F32, kind="ExternalInput")
    d_b2q = dt("b2q", [128], F32, kind="ExternalInput")
    d_w5 = dt("w5", [128, N_CLASSES], F32, kind="ExternalInput")
    d_b3q = dt("b3q", [N_CLASSES, 1], F32, kind="ExternalInput")
    d_iota60 = dt("iota60", [128, 2 * K], F32, kind="ExternalInput")
    d_off30 = dt("off30", [128, 1], F32, kind="ExternalInput")
    d_epsrow = dt("epsrow", [P, P], F32, kind="ExternalInput")
    d_id128 = dt("id128", [128, 128], F32, kind="ExternalInput")
    d_out = dt("out", [N_CLASSES, GPC], F32, kind="ExternalOutput")

    with tile.TileContext(nc) as tc:
        _emit(tc, nc, locals())
    nc.compile()
    return nc


def _ap(base, extra_offset, free_dims):
    """Build a custom AP view: keep base's partition dim, replace free dims."""
    return bass.AP(base.tensor, base.offset + extra_offset,
                   [base.ap[0]] + list(free_dims))


def _emit(tc, nc, d):
    from contextlib import ExitStack
    ctx = ExitStack()
    with ctx:
        persist = ctx.enter_context(tc.tile_pool(name="persist", bufs=1))
        act_pool = ctx.enter_context(tc.tile_pool(name="acts", bufs=1))
        w1_pool = ctx.enter_context(tc.tile_pool(name="w1p", bufs=1))

        # ---- persistent loads (weight DMAs deferred until after input DMAs)
        _deferred = []

        def load(name, shape, view=None, dram=None, dtype=F32):
            t = persist.tile(shape, dtype, tag=name)
            src = (dram if dram is not None else d[f"d_{name}"]).ap()
            if view is not None:
                src = src.rearrange(*view[0], **view[1])
            _deferred.append((t, src))
            return t

        wl, wr, bias = [], [], []
        for li in range(4):
            ki = 1 if li == 0 else 2
            wl.append(load(f"wl{li}", [128, ki, H], (["(k p) o -> p k o"], {"p": 128}),
                           dram=d["d_wl"][li]))
            wr.append(load(f"wr{li}", [128, ki, H], (["(k p) o -> p k o"], {"p": 128}),
                           dram=d["d_wr"][li]))
            bias.append(load(f"b{li}", [128, 2], (["(h p) -> p h"], {"p": 128}),
                             dram=d["d_b"][li]))
        w2 = load("w2", [128, 2 * KS, H], (["k p o -> p k o"], {}), dtype=BF16)
        cb = load("cb", [128, 2], (["(h p) -> p h"], {"p": 128}))
        b1 = load("lb1", [GPC, H])
        w4 = load("w4", [128, 2, 128], (["(k p) o -> p k o"], {"p": 128}))
        b2q = load("b2q", [128, 1])
        w5 = load("w5", [128, N_CLASSES])
        b3q = load("b3q", [N_CLASSES, 1])
        iota60 = load("iota60", [128, 2 * K])
        off30 = load("off30", [128, 1])
        epsrow = load("epsrow", [P, P])
        id128 = load("id128", [128, 128])

        # lin1 weights: first W1PRE tiles preloaded bf16 during SAGE, rest
        # streamed during the tail
        w1a = w1_pool.tile([128, W1PRE, H], BF16, tag="w1a")

        # ---- activations
        h_sb = act_pool.tile([128, PAIRS, H], F32, tag="h")       # nodes on partitions
        hT_a = act_pool.tile([128, 2, NPC], F32, tag="hTa")
        hT_b = act_pool.tile([128, 2, NPC], F32, tag="hTb")
        hts = [hT_a, hT_b]

        # ---- input DMAs: critical pieces first on each queue (sync, scalar,
        # gpsimd); x lands in h_sb[:, :, 0:128]
        with tc.tile_pool(name="sage", bufs=1) as sg:
            aggT = sg.tile([128, 2, NPC], F32, tag="aggT")
            atbd_parts = []
            for g in range(4):
                t_at = sg.tile([128, PAIRS // 4, 128], F32, tag=f"atbd{g}",
                               name=f"atbd{g}")
                atbd_parts.append(t_at)
            for g in range(4):
                qa, qb = (nc.sync, nc.scalar) if g % 2 == 0 else (nc.scalar, nc.sync)
                src = d["d_x"].ap().rearrange("(t p) f -> p t f", p=128)[:, g * 8:(g + 1) * 8, :]
                qa.dma_start(h_sb[:, g * 8:(g + 1) * 8, 0:F], src)
                srca = d["d_atbd"].ap().rearrange("t p n -> p t n")[:, g * 8:(g + 1) * 8, :]
                qb.dma_start(atbd_parts[g][...], srca)
            for g in range(4):
                (nc.sync if g % 2 == 0 else nc.scalar).dma_start(
                    hT_a[:, 0, g * 1024:(g + 1) * 1024],
                    d["d_xt"].ap()[:, g * 1024:(g + 1) * 1024])
            for _i, (_t, _src) in enumerate(_deferred):
                (nc.sync if _i % 2 == 0 else nc.scalar).dma_start(_t[...], _src)
            _deferred.clear()
            nc.gpsimd.dma_start(
                w1a[...],
                d["d_w1"].ap()[0:W1PRE].rearrange("s p o -> p s o"))

            # ---- SAGE layers, layer-phased (aggs -> weights -> transposes)
            with tc.tile_pool(name="ps_sage", bufs=2, space="PSUM") as psa, \
                 tc.tile_pool(name="ps_w", bufs=2, space="PSUM") as psw, \
                 tc.tile_pool(name="ps_tr", bufs=2, space="PSUM") as pst:
                for li in range(NLAYERS):
                    ki = 1 if li == 0 else 2
                    hTv = hts[li % 2]                    # prev layer's hT (L0: xT half)
                    hTo = hts[(li + 1) % 2]              # this layer's output

                    for t in range(PAIRS):
                        for mh in range(ki):
                            ps = psa.tile([128, 128], F32, tag="psa")
                            lhsT = (h_sb[:, t, 0:F] if li == 0
                                    else h_sb[:, t, mh * 128:(mh + 1) * 128])
                            nc.tensor.matmul(
                                ps[...],
                                lhsT=lhsT,
                                rhs=atbd_parts[t // 8][:, t % 8, :],
                                start=True, stop=True)
                            nc.any.tensor_copy(
                                aggT[:, mh, t * 128:(t + 1) * 128], ps[...])

                    for oh in range(2):
                        for c in range(NCH):
                            sl = slice(c * NCHUNK, (c + 1) * NCHUNK)
                            ps = psw.tile([128, NCHUNK], F32, tag="psw")
                            step, nsteps = 0, 2 * ki
                            for wmat, rt in ((wl[li], aggT), (wr[li], hTv)):
                                for kh in range(ki):
                                    nc.tensor.matmul(
                                        ps[...],
                                        lhsT=wmat[:, kh, oh * 128:(oh + 1) * 128],
                                        rhs=rt[:, kh, sl],
                                        start=(step == 0), stop=(step == nsteps - 1))
                                    step += 1
                            nc.scalar.activation(
                                hTo[:, oh, sl], ps[...],
                                mybir.ActivationFunctionType.Relu,
                                bias=bias[li][:, oh:oh + 1])

                    if li < NLAYERS - 1:
                        # h_next = transpose(hT_next) per pair (PE transpose)
                        for t in range(PAIRS):
                            for oh in range(2):
                                ps = pst.tile([128, 128], F32, tag="pst")
                                nc.tensor.transpose(
                                    ps[...], hTo[:, oh, t * 128:(t + 1) * 128],
                                    id128[...])
                                nc.any.tensor_copy(
                                    h_sb[:, t, oh * 128:(oh + 1) * 128], ps[...])

        # ---------------- tail: sort, selection, conv, mlp
        hT4 = hts[NLAYERS % 2]
        with tc.tile_pool(name="tail", bufs=1) as tp_:
            h3bf = tp_.tile([128, PAIRS, H], BF16, tag="h3bf")
            pt_all = tp_.tile([128, PAIRS, 2 * K], BF16, tag="pt")
            topkT = tp_.tile([128, 2, TKPAD], BF16, tag="topkT")

            with tc.tile_pool(name="ps_tr2", bufs=2, space="PSUM") as pst2:
                # ---- sort: ranks of the last feature channel per graph
                with tc.tile_pool(name="sort_scratch", bufs=1) as ss:
                    km = ss.tile([P, P], F32, tag="km")
                    # keys: feature 255 = (hi=1, p=127); node n = g*64+i
                    nc.sync.dma_start(km[...], hT4[127:128, 1, :])
                    kmp = ss.tile([P, P], F32, tag="kmp")
                    nc.vector.tensor_sub(kmp[...], km[...], epsrow[...])
                    cbt = ss.tile([P, P * P], BF16, tag="cbt")
                    kb = kmp[:, :]
                    in0 = _ap(kb, 0, [[0, P], kb.ap[1]])   # [g, i(bc), j]  k(g, j)
                    in1 = _ap(kb, 0, [kb.ap[1], [0, P]])   # [g, i, j(bc)]  k(g, i)
                    nc.vector.tensor_tensor(
                        _ap(cbt[:, :], 0, [[P, P], [1, P]]), in0, in1,
                        op=mybir.AluOpType.is_gt)
                    rk = ss.tile([P, P], F32, tag="rk")
                    nc.vector.tensor_reduce(
                        rk[...], _ap(cbt[:, :], 0, [[P, P], [1, P]]),
                        axis=mybir.AxisListType.X, op=mybir.AluOpType.add)
                    # transpose ranks -> [node i, graph g]
                    rt = ss.tile([P, P], F32, tag="rt")
                    pr = pst2.tile([P, P], F32, tag="pr")
                    nc.tensor.transpose(pr[...], rk[...], id128[0:P, 0:P])
                    nc.any.tensor_copy(rt[...], pr[...])
                    # rankP[p, t] = rank(node p%64 of graph 2t + p//64)
                    rankp = tp_.tile([128, PAIRS], F32, tag="rankp")
                    rb = rt[:, :]
                    nc.vector.tensor_copy(rankp[0:P, :], _ap(rb, 0, [[2, PAIRS]]))
                    nc.sync.dma_start(rankp[P:128, :], _ap(rb, 1, [[2, PAIRS]]))
                    # rank2 = rankp + 30*(p>=64) + 1000*(rankp>=30)
                    ge30 = tp_.tile([128, PAIRS], F32, tag="ge30")
                    nc.vector.tensor_scalar(ge30[...], rankp[...], float(K), None,
                                            op0=mybir.AluOpType.is_ge)
                    rank2 = tp_.tile([128, PAIRS], F32, tag="rank2")
                    nc.vector.scalar_tensor_tensor(
                        rank2[...], ge30[...], 1000.0,
                        rankp[...], op0=mybir.AluOpType.mult,
                        op1=mybir.AluOpType.add)
                    nc.vector.tensor_scalar(rank2[...], rank2[...],
                                            off30[:, 0:1], None,
                                            op0=mybir.AluOpType.add)
                    # one-hot selection matrices  PT[p, t, c] = (c == rank2[p, t])
                    io = iota60[:, :]
                    r2 = rank2[:, :]
                    nc.vector.tensor_tensor(
                        pt_all[...],
                        _ap(io, 0, [[0, PAIRS], [1, 2 * K]]),
                        _ap(r2, 0, [[1, PAIRS], [0, 2 * K]]),
                        op=mybir.AluOpType.is_equal)

                # ---- L3 transposes -> bf16 node-major copy for selection
                for t in range(PAIRS):
                    for oh in range(2):
                        ps = pst2.tile([128, 128], F32, tag="pst2")
                        nc.tensor.transpose(
                            ps[...], hT4[:, oh, t * 128:(t + 1) * 128], id128[...])
                        nc.any.tensor_copy(
                            h3bf[:, t, oh * 128:(oh + 1) * 128], ps[...])

            with tc.tile_pool(name="convt", bufs=1) as cv, \
                 tc.tile_pool(name="w1s", bufs=11) as w1sp:
                y_sb = cv.tile([128, 2, GPC, L28], BF16, tag="y")
                # stream the rest of the lin1 weights on the sync queue so the
                # transfers overlap selection + conv
                w1rest = []
                for s in range(W1PRE, S1):
                    w1t = w1sp.tile([128, H], BF16, tag="w1t")
                    nc.sync.dma_start(w1t[...], d["d_w1"].ap()[s])
                    w1rest.append(w1t)
                with tc.tile_pool(name="ps_tail", bufs=2, space="PSUM") as ptl:
                    # ---- selection: topkT[f, b*30+r] = sum_n h3[n,f] PT[n,b,r]
                    nc.vector.memset(topkT[:, :, GPC * K:], 0.0)
                    for t in range(PAIRS):
                        for mh in range(2):
                            ps = ptl.tile([128, 2 * K], F32, tag="pssel")
                            nc.tensor.matmul(
                                ps[...],
                                lhsT=h3bf[:, t, mh * 128:(mh + 1) * 128],
                                rhs=pt_all[:, t, :],
                                start=True, stop=True)
                            nc.any.tensor_copy(
                                topkT[:, mh, t * 2 * K:(t + 1) * 2 * K], ps[...])

                    # conv1d: y[p,oh,b,l] = relu(sum_{k,ih} w2^T topkT[...] + cb)
                    for oh in range(2):
                        for bc in range(GPC // GCHUNK):
                            ps = ptl.tile([128, GCHUNK, L28], F32, tag="psconv")
                            step = 0
                            for k in range(KS):
                                for ih in range(2):
                                    base = topkT[:, ih, :]
                                    rhs = _ap(base, bc * GCHUNK * K + k,
                                              [[K, GCHUNK], [1, L28]])
                                    nc.tensor.matmul(
                                        ps[...],
                                        lhsT=w2[:, k * 2 + ih,
                                                oh * 128:(oh + 1) * 128],
                                        rhs=rhs,
                                        start=(step == 0),
                                        stop=(step == 2 * KS - 1))
                                    step += 1
                            nc.scalar.activation(
                                y_sb[:, oh, bc * GCHUNK:(bc + 1) * GCHUNK, :],
                                ps[...],
                                mybir.ActivationFunctionType.Relu,
                                bias=cb[:, oh:oh + 1])

                with tc.tile_pool(name="ps_fin", bufs=1, space="PSUM") as pfin:
                    # lin1 (b-major): z1T[b, o] = relu(sum_s y_s^T @ w1_s + b1)
                    ps1 = pfin.tile([GPC, H], F32, tag="ps1")
                    for s in range(S1):
                        ot, l = divmod(s, L_OUT)
                        rhs = (w1a[:, s, :] if s < W1PRE
                               else w1rest[s - W1PRE][...])
                        nc.tensor.matmul(
                            ps1[...],
                            lhsT=y_sb[:, ot, :, l],
                            rhs=rhs,
                            start=(s == 0), stop=(s == S1 - 1))
                    z1t = cv.tile([GPC, H], F32, tag="z1t")
                    nc.vector.tensor_add(z1t[...], ps1[...], b1[...])
                    nc.scalar.activation(z1t[...], z1t[...],
                                         mybir.ActivationFunctionType.Relu,
                                         bias=0.0)
                    # transpose z1T -> z1 [o on partitions]
                    z1 = cv.tile([128, 2, GPC], F32, tag="z1")
                    for mh in range(2):
                        psz = pfin.tile([128, GPC], F32, tag="psz")
                        nc.tensor.transpose(psz[...],
                                            z1t[:, mh * 128:(mh + 1) * 128],
                                            id128[0:GPC, 0:GPC])
                        nc.any.tensor_copy(z1[:, mh, :], psz[...])

                    # lin2 + out
                    ps2 = pfin.tile([128, GPC], F32, tag="ps2")
                    for kh in range(2):
                        nc.tensor.matmul(ps2[...], lhsT=w4[:, kh, :],
                                         rhs=z1[:, kh, :],
                                         start=(kh == 0), stop=(kh == 1))
                    z2 = cv.tile([128, GPC], F32, tag="z2")
                    nc.scalar.activation(z2[...], ps2[...],
                                         mybir.ActivationFunctionType.Relu,
                                         bias=b2q[:, 0:1])
                    ps3 = pfin.tile([N_CLASSES, GPC], F32, tag="ps3")
                    nc.tensor.matmul(ps3[...], lhsT=w5[...], rhs=z2[...],
                                     start=True, stop=True)
                    o_sb = cv.tile([N_CLASSES, GPC], F32, tag="osb")
                    nc.scalar.activation(o_sb[...], ps3[...],
                                         mybir.ActivationFunctionType.Relu,
                                         bias=b3q[:, 0:1])
                    nc.sync.dma_start(d["d_out"].ap(), o_sb[...])


# ---------------------------------------------------------------- entry point

_CACHED = {}


def _get_nc():
    if "nc" not in _CACHED:
        nc = bacc.Bacc("TRN2", target_bir_lowering=False, debug=False,
                       enable_asserts=True)
        _CACHED["nc"] = _build(nc)
    return _CACHED["nc"]


def make_in_maps(inputs):
    sh = _prep_shared(inputs)
    cores = _prep_cores(inputs)
    return [{**sh, **c} for c in cores]


TRACE = False


def kernel(**inputs):
    in_maps = make_in_maps(inputs)
    nc = _get_nc()
    res = run_bass_kernel_spmd(nc, in_maps, core_ids=list(range(N_CORES)),
                               trace=TRACE)
    _CACHED["last_res"] = res
    return np.concatenate([np.asarray(r["out"], np.float32).T
                           for r in res.results], axis=0)


if __name__ == "__main__":
    import reference
    inputs = {k: np.asarray(v) for k, v in reference.setup_inputs().items()}
    out = kernel(**inputs)
    print("out", out.shape, out.dtype)


# revision 20
# speedup vs baseline: 1.0961x; 1.0191x over previous
"""DGCNN (4x SAGEConv + SortPool + Conv1d + MLP) Trainium2 Bass kernel.

Sharding: data-parallel over the B=512 graphs -> 64 graphs per core on 8 cores.
Edges never cross graphs, so each core's message passing is local. The edge
list is converted on the host into a per-graph normalized adjacency
(AT[g][s,d] = multiplicity(s->d) / max(deg(d),1)); aggregation then becomes a
block-diagonal dense matmul on the PE array (2 graphs of 64 nodes per
128-partition tile).

The SAGE stack runs in strict fp32: the SortPool keys (feature 255 of layer 3)
have top-30 gaps as small as 5.7e-7 on this data, so any matmul rounding on
that path (f32r keeps only ~11 mantissa bits) would reorder the selection and
blow the error gate. Everything downstream of the selection (one-hot gather,
conv1d, lin1) tolerates bf16, which runs the PE at 4x the fp32 rate.

SortPool is computed exactly (stable argsort semantics incl. ties) via a rank
computation: rank(i) = #{j : k_j > k_i} on keys perturbed by k_i -= i*1e-11.
Selection of the top-30 rows per graph is a one-hot bf16 matmul.

Conv1d is 8 accumulated bf16 matmuls per output tile (im2col via strided
access patterns, never materialized). lin1 weights are preloaded bf16.
"""

import numpy as np
import ml_dtypes

import concourse.bass as bass
import concourse.bacc as bacc
import concourse.mybir as mybir
import concourse.tile as tile
from concourse.bass_utils import run_bass_kernel_spmd

B, P, K, KS = 512, 64, 30, 4
N, E, F, H = B * P, 524288, 128, 256
L_OUT = K - KS + 1          # 27
N_CLASSES = 10
N_CORES = 8
GPC = B // N_CORES          # 64 graphs / core
NPC = GPC * P               # 4096 nodes / core
PAIRS = GPC // 2            # 32 pair-tiles (2 graphs of 64 nodes = 128 partitions)
NCHUNK = 512                # free-dim chunk for weight matmuls (4 pairs)
CPAIRS = NCHUNK // 128      # pairs per chunk
NCH = NPC // NCHUNK         # 8 chunks
W1PRE = 32                  # lin1 weight tiles preloaded during SAGE (rest streamed)
F32 = mybir.dt.float32
BF16 = mybir.dt.bfloat16
EPS_TIE = 1e-11

NLAYERS = 4
GCHUNK = 16                 # graphs per conv psum tile (16*28 = 448 <= 512)
L28 = L_OUT + 1             # conv free dim padded even
TKPAD = GPC * K + 8         # topkT free size incl. zeroed overrun pad
S1 = 2 * L_OUT              # 54 lin1 contraction steps of 128
BF = np.dtype(ml_dtypes.bfloat16)


# ---------------------------------------------------------------- host prep

def _prep_shared(inp):
    """Host-side weight/constant reshaping (identical for every core)."""
    sh = {}
    for li in range(4):
        ki = 1 if li == 0 else 2
        # partition-major [p, k, o] so the DMA is contiguous
        sh[f"wl{li}"] = np.ascontiguousarray(
            np.asarray(inp[f"sage{li}_wl"], np.float32)
            .reshape(ki, 128, H).transpose(1, 0, 2))
        sh[f"wr{li}"] = np.ascontiguousarray(
            np.asarray(inp[f"sage{li}_wr"], np.float32)
            .reshape(ki, 128, H).transpose(1, 0, 2))
        sh[f"b{li}"] = np.ascontiguousarray(
            np.asarray(inp[f"sage{li}_b"], np.float32).reshape(2, 128).T)
    w = np.asarray(inp["conv1d_w"], np.float32)            # [O=256, I=256, KS]
    w2 = np.empty((2 * KS, 128, H), np.float32)
    for k in range(KS):
        wt = w[:, :, k].T                                  # [I, O]
        for ih in range(2):
            w2[k * 2 + ih] = wt[ih * 128:(ih + 1) * 128]
    sh["w2"] = np.ascontiguousarray(w2.transpose(1, 0, 2)).astype(BF)
    sh["cb"] = np.ascontiguousarray(
        np.asarray(inp["conv1d_b"], np.float32).reshape(2, 128).T)
    w1 = np.asarray(inp["lin1_w"], np.float32)             # [6912, 256]
    sh["w1"] = np.ascontiguousarray(
        w1.reshape(2, 128, L_OUT, H).transpose(0, 2, 1, 3).reshape(S1, 128, H)
        .transpose(1, 0, 2)).astype(BF)                    # [128, S1, H]
    sh["lb1"] = np.ascontiguousarray(
        np.broadcast_to(np.asarray(inp["lin1_b"], np.float32), (GPC, H)))
    sh["w4"] = np.ascontiguousarray(
        np.asarray(inp["lin2_w"], np.float32)
        .reshape(2, 128, 128).transpose(1, 0, 2))            # [128, 2, 128]
    sh["b2q"] = np.ascontiguousarray(inp["lin2_b"], np.float32)  # [128]
    sh["w5"] = np.ascontiguousarray(inp["out_w"], np.float32)    # [128, 10]
    sh["b3q"] = np.asarray(inp["out_b"], np.float32).reshape(N_CLASSES, 1).copy()
    sh["iota60"] = np.ascontiguousarray(
        np.broadcast_to(np.arange(2 * K, dtype=np.float32), (128, 2 * K)))
    off30 = np.zeros((128, 1), np.float32)
    off30[64:] = float(K)
    sh["off30"] = off30
    sh["epsrow"] = np.ascontiguousarray(
        np.broadcast_to(np.arange(P, dtype=np.float32) * np.float32(EPS_TIE), (P, P))).astype(np.float32)
    sh["id128"] = np.eye(128, dtype=np.float32)
    return sh


def _prep_cores(inp):
    """Per-core shards: node features (plain + transposed) and blockdiag adjacency."""
    x = np.nan_to_num(np.asarray(inp["x"], np.float32))
    ei = np.asarray(inp["edge_index"])
    src = ei[0].astype(np.int64)
    dst = ei[1].astype(np.int64)
    deg = np.bincount(dst, minlength=N).astype(np.float32)
    inv_deg = (1.0 / np.maximum(deg, 1.0)).astype(np.float32)
    g = src // P
    flat = g * (P * P) + (src % P) * P + (dst % P)
    AT = np.bincount(flat, minlength=B * P * P).astype(np.float32).reshape(B, P, P)
    AT *= inv_deg.reshape(B, P)[:, None, :]

    cores = []
    for c in range(N_CORES):
        xc = np.ascontiguousarray(x[c * NPC:(c + 1) * NPC])          # [4096, 128]
        atbd = np.zeros((PAIRS, 128, 128), np.float32)
        for t in range(PAIRS):
            atbd[t, :P, :P] = AT[c * GPC + 2 * t]
            atbd[t, P:, P:] = AT[c * GPC + 2 * t + 1]
        cores.append({
            # partition-major [p, t, f] / [p, t, n] so DMAs are contiguous
            "x": np.ascontiguousarray(
                xc.reshape(PAIRS, 128, F).transpose(1, 0, 2)),       # [128, 32, 128]
            "xt": np.ascontiguousarray(xc.T),                        # [128, 4096]
            "atbd": np.ascontiguousarray(atbd.transpose(1, 0, 2)),   # [128, 32, 128]
        })
    return cores


# ---------------------------------------------------------------- device kernel

def _build(nc):
    """Emit the whole per-core kernel under a TileContext."""
    dt = nc.dram_tensor
    d_x = dt("x", [128, PAIRS, F], F32, kind="ExternalInput")
    d_xt = dt("xt", [F, NPC], F32, kind="ExternalInput")
    d_atbd = dt("atbd", [128, PAIRS, 128], F32, kind="ExternalInput")
    d_wl, d_wr, d_b = [], [], []
    for li in range(4):
        ki = 1 if li == 0 else 2
        d_wl.append(dt(f"wl{li}", [128, ki, H], F32, kind="ExternalInput"))
        d_wr.append(dt(f"wr{li}", [128, ki, H], F32, kind="ExternalInput"))
        d_b.append(dt(f"b{li}", [128, 2], F32, kind="ExternalInput"))
    d_w2 = dt("w2", [128, 2 * KS, H], BF16, kind="ExternalInput")
    d_cb = dt("cb", [128, 2], F32, kind="ExternalInput")
    d_w1 = dt("w1", [128, S1, H], BF16, kind="ExternalInput")
    d_lb1 = dt("lb1", [GPC, H], F32, kind="ExternalInput")
    d_w4 = dt("w4", [128, 2, 128], F32, kind="ExternalInput")
    d_b2q = dt("b2q", [128], F32, kind="ExternalInput")
    d_w5 = dt("w5", [128, N_CLASSES], F32, kind="ExternalInput")
    d_b3q = dt("b3q", [N_CLASSES, 1], F32, kind="ExternalInput")
    d_iota60 = dt("iota60", [128, 2 * K], F32, kind="ExternalInput")
    d_off30 = dt("off30", [128, 1], F32, kind="ExternalInput")
    d_epsrow = dt("epsrow", [P, P], F32, kind="ExternalInput")
    d_id128 = dt("id128", [128, 128], F32, kind="ExternalInput")
    d_out = dt("out", [N_CLASSES, GPC], F32, kind="ExternalOutput")

    with tile.TileContext(nc) as tc:
        _emit(tc, nc, locals())
    nc.compile()
    return nc


def _ap(base, extra_offset, free_dims):
    """Build a custom AP view: keep base's partition dim, replace free dims."""
    return bass.AP(base.tensor, base.offset + extra_offset,
                   [base.ap[0]] + list(free_dims))


def _emit(tc, nc, d):
    from contextlib import ExitStack
    ctx = ExitStack()
    with ctx:
        persist = ctx.enter_context(tc.tile_pool(name="persist", bufs=1))
        act_pool = ctx.enter_context(tc.tile_pool(name="acts", bufs=1))
        w1_pool = ctx.enter_context(tc.tile_pool(name="w1p", bufs=1))

        # ---- persistent loads (weight DMAs deferred until after input DMAs)
        _deferred = []

        def load(name, shape, view=None, dram=None, dtype=F32):
            t = persist.tile(shape, dtype, tag=name)
            src = (dram if dram is not None else d[f"d_{name}"]).ap()
            if view is not None:
                src = src.rearrange(*view[0], **view[1])
            _deferred.append((t, src))
            return t

        wl, wr, bias = [], [], []
        for li in range(4):
            ki = 1 if li == 0 else 2
            wl.append(load(f"wl{li}", [128, ki, H], dram=d["d_wl"][li]))
            wr.append(load(f"wr{li}", [128, ki, H], dram=d["d_wr"][li]))
            bias.append(load(f"b{li}", [128, 2], dram=d["d_b"][li]))
        w2 = load("w2", [128, 2 * KS, H], dtype=BF16)
        cb = load("cb", [128, 2])
        b1 = load("lb1", [GPC, H])
        w4 = load("w4", [128, 2, 128])
        b2q = load("b2q", [128, 1])
        w5 = load("w5", [128, N_CLASSES])
        b3q = load("b3q", [N_CLASSES, 1])
        iota60 = load("iota60", [128, 2 * K])
        off30 = load("off30", [128, 1])
        epsrow = load("epsrow", [P, P])
        id128 = load("id128", [128, 128])

        # lin1 weights: first W1PRE tiles preloaded bf16 during SAGE, rest
        # streamed during the tail
        w1a = w1_pool.tile([128, W1PRE, H], BF16, tag="w1a")

        # ---- activations
        h_sb = act_pool.tile([128, PAIRS, H], F32, tag="h")       # nodes on partitions
        hT_a = act_pool.tile([128, 2, NPC], F32, tag="hTa")
        hT_b = act_pool.tile([128, 2, NPC], F32, tag="hTb")
        hts = [hT_a, hT_b]

        # ---- input DMAs: critical pieces first on each queue (sync, scalar,
        # gpsimd); x lands in h_sb[:, :, 0:128]
        with tc.tile_pool(name="sage", bufs=1) as sg:
            aggT = sg.tile([128, 2, NPC], F32, tag="aggT")
            atbd_parts = []
            for g in range(4):
                t_at = sg.tile([128, PAIRS // 4, 128], F32, tag=f"atbd{g}",
                               name=f"atbd{g}")
                atbd_parts.append(t_at)
            for g in range(4):
                qa, qb = (nc.sync, nc.scalar) if g % 2 == 0 else (nc.scalar, nc.sync)
                qa.dma_start(h_sb[:, g * 8:(g + 1) * 8, 0:F],
                             d["d_x"].ap()[:, g * 8:(g + 1) * 8, :])
                qb.dma_start(atbd_parts[g][...],
                             d["d_atbd"].ap()[:, g * 8:(g + 1) * 8, :])
            for g in range(4):
                (nc.sync if g % 2 == 0 else nc.scalar).dma_start(
                    hT_a[:, 0, g * 1024:(g + 1) * 1024],
                    d["d_xt"].ap()[:, g * 1024:(g + 1) * 1024])
            for _i, (_t, _src) in enumerate(_deferred):
                (nc.sync if _i % 2 == 0 else nc.scalar).dma_start(_t[...], _src)
            _deferred.clear()
            nc.gpsimd.dma_start(w1a[...], d["d_w1"].ap()[:, 0:W1PRE, :])

            # ---- SAGE layers, layer-phased (aggs -> weights -> transposes)
            with tc.tile_pool(name="ps_sage", bufs=2, space="PSUM") as psa, \
                 tc.tile_pool(name="ps_w", bufs=2, space="PSUM") as psw, \
                 tc.tile_pool(name="ps_tr", bufs=2, space="PSUM") as pst:
                for li in range(NLAYERS):
                    ki = 1 if li == 0 else 2
                    hTv = hts[li % 2]                    # prev layer's hT (L0: xT half)
                    hTo = hts[(li + 1) % 2]              # this layer's output

                    for t in range(PAIRS):
                        for mh in range(ki):
                            ps = psa.tile([128, 128], F32, tag="psa")
                            lhsT = (h_sb[:, t, 0:F] if li == 0
                                    else h_sb[:, t, mh * 128:(mh + 1) * 128])
                            nc.tensor.matmul(
                                ps[...],
                                lhsT=lhsT,
                                rhs=atbd_parts[t // 8][:, t % 8, :],
                                start=True, stop=True)
                            nc.any.tensor_copy(
                                aggT[:, mh, t * 128:(t + 1) * 128], ps[...])

                    for oh in range(2):
                        for c in range(NCH):
                            sl = slice(c * NCHUNK, (c + 1) * NCHUNK)
                            ps = psw.tile([128, NCHUNK], F32, tag="psw")
                            step, nsteps = 0, 2 * ki
                            for wmat, rt in ((wl[li], aggT), (wr[li], hTv)):
                                for kh in range(ki):
                                    nc.tensor.matmul(
                                        ps[...],
                                        lhsT=wmat[:, kh, oh * 128:(oh + 1) * 128],
                                        rhs=rt[:, kh, sl],
                                        start=(step == 0), stop=(step == nsteps - 1))
                                    step += 1
                            nc.scalar.activation(
                                hTo[:, oh, sl], ps[...],
                                mybir.ActivationFunctionType.Relu,
                                bias=bias[li][:, oh:oh + 1])

                    if li < NLAYERS - 1:
                        # h_next = transpose(hT_next) per pair (PE transpose)
                        for t in range(PAIRS):
                            for oh in range(2):
                                ps = pst.tile([128, 128], F32, tag="pst")
                                nc.tensor.transpose(
                                    ps[...], hTo[:, oh, t * 128:(t + 1) * 128],
                                    id128[...])
                                nc.any.tensor_copy(
                                    h_sb[:, t, oh * 128:(oh + 1) * 128], ps[...])

        # ---------------- tail: sort, selection, conv, mlp
        hT4 = hts[NLAYERS % 2]
        with tc.tile_pool(name="tail", bufs=1) as tp_:
            h3bf = tp_.tile([128, PAIRS, H], BF16, tag="h3bf")
            pt_all = tp_.tile([128, PAIRS, 2 * K], BF16, tag="pt")
            topkT = tp_.tile([128, 2, TKPAD], BF16, tag="topkT")

            with tc.tile_pool(name="ps_tr2", bufs=2, space="PSUM") as pst2:
                # ---- sort: ranks of the last feature channel per graph
                with tc.tile_pool(name="sort_scratch", bufs=1) as ss:
                    km = ss.tile([P, P], F32, tag="km")
                    # keys: feature 255 = (hi=1, p=127); node n = g*64+i
                    nc.sync.dma_start(km[...], hT4[127:128, 1, :])
                    kmp = ss.tile([P, P], F32, tag="kmp")
                    nc.vector.tensor_sub(kmp[...], km[...], epsrow[...])
                    cbt = ss.tile([P, P * P], BF16, tag="cbt")
                    kb = kmp[:, :]
                    in0 = _ap(kb, 0, [[0, P], kb.ap[1]])   # [g, i(bc), j]  k(g, j)
                    in1 = _ap(kb, 0, [kb.ap[1], [0, P]])   # [g, i, j(bc)]  k(g, i)
                    nc.vector.tensor_tensor(
                        _ap(cbt[:, :], 0, [[P, P], [1, P]]), in0, in1,
                        op=mybir.AluOpType.is_gt)
                    rk = ss.tile([P, P], F32, tag="rk")
                    nc.vector.tensor_reduce(
                        rk[...], _ap(cbt[:, :], 0, [[P, P], [1, P]]),
                        axis=mybir.AxisListType.X, op=mybir.AluOpType.add)
                    # transpose ranks -> [node i, graph g]
                    rt = ss.tile([P, P], F32, tag="rt")
                    pr = pst2.tile([P, P], F32, tag="pr")
                    nc.tensor.transpose(pr[...], rk[...], id128[0:P, 0:P])
                    nc.any.tensor_copy(rt[...], pr[...])
                    # rankP[p, t] = rank(node p%64 of graph 2t + p//64)
                    rankp = tp_.tile([128, PAIRS], F32, tag="rankp")
                    rb = rt[:, :]
                    nc.vector.tensor_copy(rankp[0:P, :], _ap(rb, 0, [[2, PAIRS]]))
                    nc.sync.dma_start(rankp[P:128, :], _ap(rb, 1, [[2, PAIRS]]))
                    # rank2 = rankp + 30*(p>=64) + 1000*(rankp>=30)
                    ge30 = tp_.tile([128, PAIRS], F32, tag="ge30")
                    nc.vector.tensor_scalar(ge30[...], rankp[...], float(K), None,
                                            op0=mybir.AluOpType.is_ge)
                    rank2 = tp_.tile([128, PAIRS], F32, tag="rank2")
                    nc.vector.scalar_tensor_tensor(
                        rank2[...], ge30[...], 1000.0,
                        rankp[...], op0=mybir.AluOpType.mult,
                        op1=mybir.AluOpType.add)
                    nc.vector.tensor_scalar(rank2[...], rank2[...],
                                            off30[:, 0:1], None,
                                            op0=mybir.AluOpType.add)
                    # one-hot selection matrices  PT[p, t, c] = (c == rank2[p, t])
                    io = iota60[:, :]
                    r2 = rank2[:, :]
                    nc.vector.tensor_tensor(
                        pt_all[...],
                        _ap(io, 0, [[0, PAIRS], [1, 2 * K]]),
                        _ap(r2, 0, [[1, PAIRS], [0, 2 * K]]),
                        op=mybir.AluOpType.is_equal)

                # ---- L3 transposes -> bf16 node-major copy for selection
                for t in range(PAIRS):
                    for oh in range(2):
                        ps = pst2.tile([128, 128], F32, tag="pst2")
                        nc.tensor.transpose(
                            ps[...], hT4[:, oh, t * 128:(t + 1) * 128], id128[...])
                        nc.any.tensor_copy(
                            h3bf[:, t, oh * 128:(oh + 1) * 128], ps[...])

            with tc.tile_pool(name="convt", bufs=1) as cv, \
                 tc.tile_pool(name="w1s", bufs=11) as w1sp:
                y_sb = cv.tile([128, 2, GPC, L28], BF16, tag="y")
                # stream the rest of the lin1 weights on the sync queue so the
                # transfers overlap selection + conv
                w1rest = []
                for s in range(W1PRE, S1):
                    w1t = w1sp.tile([128, H], BF16, tag="w1t")
                    ((nc.sync if s % 2 == 0 else nc.scalar)
                     .dma_start(w1t[...], d["d_w1"].ap()[:, s, :]))
                    w1rest.append(w1t)
                with tc.tile_pool(name="ps_tail", bufs=2, space="PSUM") as ptl:
                    # ---- selection: topkT[f, b*30+r] = sum_n h3[n,f] PT[n,b,r]
                    nc.vector.memset(topkT[:, :, GPC * K:], 0.0)
                    for t in range(PAIRS):
                        for mh in range(2):
                            ps = ptl.tile([128, 2 * K], F32, tag="pssel")
                            nc.tensor.matmul(
                                ps[...],
                                lhsT=h3bf[:, t, mh * 128:(mh + 1) * 128],
                                rhs=pt_all[:, t, :],
                                start=True, stop=True)
                            nc.any.tensor_copy(
                                topkT[:, mh, t * 2 * K:(t + 1) * 2 * K], ps[...])

                    # conv1d: y[p,oh,b,l] = relu(sum_{k,ih} w2^T topkT[...] + cb)
                    for oh in range(2):
                        for bc in range(GPC // GCHUNK):
                            ps = ptl.tile([128, GCHUNK, L28], F32, tag="psconv")
                            step = 0
                            for k in range(KS):
                                for ih in range(2):
                                    base = topkT[:, ih, :]
                                    rhs = _ap(base, bc * GCHUNK * K + k,
                                              [[K, GCHUNK], [1, L28]])
                                    nc.tensor.matmul(
                                        ps[...],
                                        lhsT=w2[:, k * 2 + ih,
                                                oh * 128:(oh + 1) * 128],
                                        rhs=rhs,
                                        start=(step == 0),
                                        stop=(step == 2 * KS - 1))
                                    step += 1
                            nc.scalar.activation(
                                y_sb[:, oh, bc * GCHUNK:(bc + 1) * GCHUNK, :],
                                ps[...],
                                mybir.ActivationFunctionType.Relu,
                                bias=cb[:, oh:oh + 1])

                with tc.tile_pool(name="ps_fin", bufs=1, space="PSUM") as pfin:
                    # lin1 (b-major): z1T[b, o] = relu(sum_s y_s^T @ w1_s + b1)
                    ps1 = pfin.tile([GPC, H], F32, tag="ps1")
                    for s in range(S1):
                        ot, l = divmod(s, L_OUT)
                        rhs = (w1a[:, s, :] if s < W1PRE
                               else w1rest[s - W1PRE][...])
                        nc.tensor.matmul(
                            ps1[...],
                            lhsT=y_sb[:, ot, :, l],
                            rhs=rhs,
                            start=(s == 0), stop=(s == S1 - 1))
                    z1t = cv.tile([GPC, H], F32, tag="z1t")
                    nc.vector.tensor_add(z1t[...], ps1[...], b1[...])
                    nc.scalar.activation(z1t[...], z1t[...],
                                         mybir.ActivationFunctionType.Relu,
                                         bias=0.0)
                    # transpose z1T -> z1 [o on partitions]
                    z1 = cv.tile([128, 2, GPC], F32, tag="z1")
                    for mh in range(2):
                        psz = pfin.tile([128, GPC], F32, tag="psz")
                        nc.tensor.transpose(psz[...],
                                            z1t[:, mh * 128:(mh + 1) * 128],
                                            id128[0:GPC, 0:GPC])
                        nc.any.tensor_copy(z1[:, mh, :], psz[...])

                    # lin2 + out
                    ps2 = pfin.tile([128, GPC], F32, tag="ps2")
                    for kh in range(2):
                        nc.tensor.matmul(ps2[...], lhsT=w4[:, kh, :],
                                         rhs=z1[:, kh, :],
                                         start=(kh == 0), stop=(kh == 1))
                    z2 = cv.tile([128, GPC], F32, tag="z2")
                    nc.scalar.activation(z2[...], ps2[...],
                                         mybir.ActivationFunctionType.Relu,
                                         bias=b2q[:, 0:1])
                    ps3 = pfin.tile([N_CLASSES, GPC], F32, tag="ps3")
                    nc.tensor.matmul(ps3[...], lhsT=w5[...], rhs=z2[...],
                                     start=True, stop=True)
                    o_sb = cv.tile([N_CLASSES, GPC], F32, tag="osb")
                    nc.scalar.activation(o_sb[...], ps3[...],
                                         mybir.ActivationFunctionType.Relu,
                                         bias=b3q[:, 0:1])
                    nc.sync.dma_start(d["d_out"].ap(), o_sb[...])


# ---------------------------------------------------------------- entry point

_CACHED = {}


def _get_nc():
    if "nc" not in _CACHED:
        nc = bacc.Bacc("TRN2", target_bir_lowering=False, debug=False,
                       enable_asserts=True)
        _CACHED["nc"] = _build(nc)
    return _CACHED["nc"]


def make_in_maps(inputs):
    sh = _prep_shared(inputs)
    cores = _prep_cores(inputs)
    return [{**sh, **c} for c in cores]


TRACE = False


def kernel(**inputs):
    in_maps = make_in_maps(inputs)
    nc = _get_nc()
    res = run_bass_kernel_spmd(nc, in_maps, core_ids=list(range(N_CORES)),
                               trace=TRACE)
    _CACHED["last_res"] = res
    return np.concatenate([np.asarray(r["out"], np.float32).T
                           for r in res.results], axis=0)


if __name__ == "__main__":
    import reference
    inputs = {k: np.asarray(v) for k, v in reference.setup_inputs().items()}
    out = kernel(**inputs)
    print("out", out.shape, out.dtype)
